# revision 30
# baseline (speedup 1.0000x reference)
"""Expert-parallel MoE kernel for Trainium2 (8 NeuronCores).

Strategy (expert-parallel, per sharding hint):
  - Host: sort the T*top_k dispatch pairs by expert, scale each dispatched
    token by its gate score (gate folds into the linear map's input), pad
    each expert's token group to a fixed capacity CAP, lay out as [K, M]
    (pre-transposed for the PE's lhsT operand), cast to bf16.
  - Device (SPMD, core m owns experts 2m and 2m+1): Z_e = X_e^T.T @ W_e
    as tiled bf16 matmuls with fp32 PSUM accumulation.
  - Host: scatter Z rows back to dispatch pairs, sum top_k contributions,
    add the (gate-weighted) expert biases.
"""

import numpy as np
import ml_dtypes

NUM_EXPERT = 16
D = 1024
TOP_K = 2
T = 2048
N_CORES = 8
EPC = NUM_EXPERT // N_CORES  # experts per core
CAP = 256                    # per-expert dispatch capacity (multiple of 128)
KT = D // 128                # contraction tiles
NT = D // 512                # output free-dim tiles (one PSUM bank each)
MT = CAP // 128              # token tiles

TRACE = False                # set by test harness to collect an NTFF profile
LAST_RESULT = None           # BassKernelResults of the most recent run

_NC = None


def _build_nc():
    from concourse import bacc, tile
    import concourse.mybir as mybir

    bf16 = mybir.dt.bfloat16
    f32 = mybir.dt.float32

    nc = bacc.Bacc("TRN2", target_bir_lowering=False, debug=False,
                   num_devices=N_CORES)
    # Flat chunk-major layouts: each DMA chunk is a [128, f] block whose
    # per-partition data is contiguous in DRAM (multi-KB descriptors).
    w = nc.declare_dram_parameter("w", [EPC, KT * 128 * D], bf16,
                                  isOutput=False)
    xt = nc.declare_dram_parameter("xt", [EPC, KT * 128 * CAP], bf16,
                                   isOutput=False)
    z = nc.declare_dram_parameter("z", [EPC, CAP, D], bf16, isOutput=True)

    XH = KT // 2      # k-tiles per x chunk

    with tile.TileContext(nc, num_cores=N_CORES) as tc:
        with (
            tc.tile_pool(name="wp", bufs=1) as wp,
            tc.tile_pool(name="xp", bufs=1) as xp,
            tc.tile_pool(name="pp", bufs=2, space="PSUM") as pp,
            tc.tile_pool(name="op", bufs=4) as op,
        ):
            # all loads on the sync HWDGE ring, in k order (x half, then
            # the W k-pairs it feeds); 4KB descriptors throughout
            wts, xts = {}, {}
            for e in range(EPC):
                xtt = xp.tile([128, KT * CAP], bf16,
                              name=f"x{e}", tag=f"x{e}")
                nc.sync.dma_start(
                    xtt[:],
                    xt[e].rearrange("(p f) -> p f", p=128))
                xts[e] = xtt
                for (k0, kl) in [(0, 2), (2, 2), (4, 2), (6, 2)]:
                    wtl = wp.tile([128, kl * D], bf16,
                                  name=f"w{e}_{k0}", tag=f"w{e}_{k0}")
                    src = w[e][k0 * 128 * D:(k0 + kl) * 128 * D]
                    nc.sync.dma_start(
                        wtl[:], src.rearrange("(p f) -> p f", p=128))
                    for kk in range(kl):
                        wts[e, k0 + kk] = (wtl, kk)

            # k-outer: all 4 (m, n) PSUM groups of an expert accumulate
            # in parallel, so the PE consumes each k chunk as it lands;
            # 4 banks/expert * bufs=2 = 8 banks -> experts double-buffer
            for e in range(EPC):
                pss = {}
                for m in range(MT):
                    for n in range(NT):
                        pss[m, n] = pp.tile([128, 512], f32,
                                            name=f"ps{m}{n}",
                                            tag=f"ps{m}{n}")
                for k in range(KT):
                    xtile = xts[e]
                    xoff = k * CAP
                    wtl, kk = wts[e, k]
                    for m in range(MT):
                        for n in range(NT):
                            nc.tensor.matmul(
                                pss[m, n][:],
                                xtile[:, xoff + m * 128:
                                      xoff + (m + 1) * 128],
                                wtl[:, kk * D + n * 512:
                                    kk * D + (n + 1) * 512],
                                start=(k == 0),
                                stop=(k == KT - 1),
                            )
                for m in range(MT):
                    ot = op.tile([128, D], bf16)
                    for n in range(NT):
                        nc.any.tensor_copy(
                            ot[:, n * 512:(n + 1) * 512], pss[m, n][:])
                    # e0 stores queue on the sync ring BEHIND all loads
                    # (ring FIFO keeps store traffic out of the critical
                    # load window); the two final stores go in parallel
                    # on separate rings
                    eng = nc.scalar if (e, m) == (EPC - 1, 0) else nc.sync
                    eng.dma_start(
                        z[e, m * 128:(m + 1) * 128, :], ot[:])
    nc.compile()
    return nc


def kernel(inp, gate_idx, gate_score, W, b):
    global _NC, LAST_RESULT
    from concourse.bass_utils import run_bass_kernel_spmd

    inp = np.ascontiguousarray(np.asarray(inp, dtype=np.float32))
    gi = np.asarray(gate_idx).astype(np.int64)
    gs = np.asarray(gate_score, dtype=np.float32)
    W = np.asarray(W, dtype=np.float32)
    b = np.asarray(b, dtype=np.float32)

    P = T * TOP_K
    fe = gi.reshape(P)
    fg = gs.reshape(P)
    tok = np.arange(P) // TOP_K

    order = np.argsort(fe, kind="stable")
    counts = np.bincount(fe, minlength=NUM_EXPERT)
    starts = np.zeros(NUM_EXPERT + 1, np.int64)
    np.cumsum(counts, out=starts[1:])
    rank = np.arange(P) - starts[fe[order]]
    ok = rank < CAP
    sel = order[ok]
    rnk = rank[ok]

    xpad = np.zeros((NUM_EXPERT, CAP, D), np.float32)
    xpad[fe[sel], rnk] = inp[tok[sel]] * fg[sel, None]
    # flat chunk-major device layouts (must match _build_nc's chunking):
    # each chunk is [128, kl*D] with per-partition data contiguous
    xk = xpad.reshape(NUM_EXPERT, CAP, KT, 128)
    xt_dev = np.ascontiguousarray(
        xk.transpose(0, 3, 2, 1)
    ).reshape(NUM_EXPERT, -1).astype(ml_dtypes.bfloat16)
    wk = W.reshape(NUM_EXPERT, KT, 128, D)
    w_parts = [
        np.ascontiguousarray(
            wk[:, k0:k0 + kl].transpose(0, 2, 1, 3)
        ).reshape(NUM_EXPERT, -1)
        for (k0, kl) in [(0, 2), (2, 2), (4, 2), (6, 2)]
    ]
    w_dev = np.concatenate(w_parts, axis=1).astype(ml_dtypes.bfloat16)

    if _NC is None:
        _NC = _build_nc()

    in_maps = [
        {"w": w_dev[c * EPC:(c + 1) * EPC],
         "xt": xt_dev[c * EPC:(c + 1) * EPC]}
        for c in range(N_CORES)
    ]
    res = run_bass_kernel_spmd(_NC, in_maps, list(range(N_CORES)),
                               trace=TRACE)
    LAST_RESULT = res
    zall = np.concatenate(
        [np.asarray(r["z"]).astype(np.float32) for r in res.results],
        axis=0)  # [E,CAP,D]

    zpairs = np.zeros((P, D), np.float32)
    zpairs[sel] = zall[fe[sel], rnk]
    # exact f32 fallback for over-capacity pairs (~2% of dispatches)
    overflow = order[~ok]
    if overflow.size:
        fe_o = fe[overflow]
        for e in np.unique(fe_o):
            pi = overflow[fe_o == e]
            zpairs[pi] = (inp[tok[pi]] * fg[pi, None]) @ W[e]

    y = zpairs.reshape(T, TOP_K, D).sum(axis=1)
    y += (gs[:, :, None] * b[gi]).sum(axis=1)
    return y.astype(np.float32)


# revision 33
# speedup vs baseline: 1.2968x; 1.2968x over previous
"""Expert-parallel MoE kernel for Trainium2 (8 NeuronCores).

Strategy (expert-parallel, per sharding hint):
  - Host: sort the T*top_k dispatch pairs by expert, scale each dispatched
    token by its gate score (gate folds into the linear map's input), pad
    each expert's token group to a fixed capacity CAP, lay out as [K, M]
    (pre-transposed for the PE's lhsT operand), cast to bf16.
  - Device (SPMD, core m owns experts 2m and 2m+1): Z_e = X_e^T.T @ W_e
    as tiled bf16 matmuls with fp32 PSUM accumulation.
  - Host: scatter Z rows back to dispatch pairs, sum top_k contributions,
    add the (gate-weighted) expert biases.
"""

import numpy as np
import ml_dtypes

NUM_EXPERT = 16
D = 1024
TOP_K = 2
T = 2048
N_CORES = 8
EPC = NUM_EXPERT // N_CORES  # experts per core
CAP = 256                    # per-expert dispatch capacity (multiple of 128)
KT = D // 128                # contraction tiles
NT = D // 512                # output free-dim tiles (one PSUM bank each)
MT = CAP // 128              # token tiles

TRACE = False                # set by test harness to collect an NTFF profile
LAST_RESULT = None           # BassKernelResults of the most recent run

_NC = None


def _build_nc():
    from concourse import bacc, tile
    import concourse.mybir as mybir

    bf16 = mybir.dt.bfloat16
    f32 = mybir.dt.float32

    nc = bacc.Bacc("TRN2", target_bir_lowering=False, debug=False,
                   num_devices=N_CORES)
    # Flat chunk-major layouts: each DMA chunk is a [128, f] block whose
    # per-partition data is contiguous in DRAM (multi-KB descriptors).
    w = nc.declare_dram_parameter("w", [EPC, KT * 128 * D], bf16,
                                  isOutput=False)
    xt = nc.declare_dram_parameter("xt", [EPC, KT * 128 * CAP], bf16,
                                   isOutput=False)
    z = nc.declare_dram_parameter("z", [EPC, CAP, D], bf16, isOutput=True)

    XH = KT // 2      # k-tiles per x chunk

    with tile.TileContext(nc, num_cores=N_CORES) as tc:
        with (
            tc.tile_pool(name="wp", bufs=1) as wp,
            tc.tile_pool(name="xp", bufs=1) as xp,
            tc.tile_pool(name="pp", bufs=2, space="PSUM") as pp,
            tc.tile_pool(name="op", bufs=4) as op,
        ):
            # all loads on the sync HWDGE ring, in k order (x half, then
            # the W k-pairs it feeds); 4KB descriptors throughout
            wts, xts = {}, {}
            for e in range(EPC):
                xts[e] = []
                for h in range(2):
                    xtt = xp.tile([128, XH * CAP], bf16,
                                  name=f"x{e}_{h}", tag=f"x{e}_{h}")
                    src = xt[e][h * (XH * 128 * CAP):
                               (h + 1) * (XH * 128 * CAP)]
                    nc.sync.dma_start(
                        xtt[:], src.rearrange("(p f) -> p f", p=128))
                    xts[e].append(xtt)
                    for (k0, kl) in [(h * XH, 2), (h * XH + 2, 2)]:
                        wtl = wp.tile([128, kl * D], bf16,
                                      name=f"w{e}_{k0}", tag=f"w{e}_{k0}")
                        src = w[e][k0 * 128 * D:(k0 + kl) * 128 * D]
                        nc.sync.dma_start(
                            wtl[:], src.rearrange("(p f) -> p f", p=128))
                        for kk in range(kl):
                            wts[e, k0 + kk] = (wtl, kk)

            # k-outer: all 4 (m, n) PSUM groups of an expert accumulate
            # in parallel, so the PE consumes each k chunk as it lands;
            # 4 banks/expert * bufs=2 = 8 banks -> experts double-buffer
            for e in range(EPC):
                pss = {}
                for m in range(MT):
                    for n in range(NT):
                        pss[m, n] = pp.tile([128, 512], f32,
                                            name=f"ps{m}{n}",
                                            tag=f"ps{m}{n}")
                for k in range(KT):
                    xtile = xts[e][k // XH]
                    xoff = (k % XH) * CAP
                    wtl, kk = wts[e, k]
                    for m in range(MT):
                        for n in range(NT):
                            nc.tensor.matmul(
                                pss[m, n][:],
                                xtile[:, xoff + m * 128:
                                      xoff + (m + 1) * 128],
                                wtl[:, kk * D + n * 512:
                                    kk * D + (n + 1) * 512],
                                start=(k == 0),
                                stop=(k == KT - 1),
                            )
                for m in range(MT):
                    ot = op.tile([128, D], bf16)
                    for n in range(NT):
                        nc.any.tensor_copy(
                            ot[:, n * 512:(n + 1) * 512], pss[m, n][:])
                    # e0 stores queue on the sync ring BEHIND all loads
                    # (ring FIFO keeps store traffic out of the critical
                    # load window); the two final stores go in parallel
                    # on separate rings
                    eng = nc.scalar if (e, m) == (EPC - 1, 0) else nc.sync
                    eng.dma_start(
                        z[e, m * 128:(m + 1) * 128, :], ot[:])
    nc.compile()
    return nc


def kernel(inp, gate_idx, gate_score, W, b):
    global _NC, LAST_RESULT
    from concourse.bass_utils import run_bass_kernel_spmd

    inp = np.ascontiguousarray(np.asarray(inp, dtype=np.float32))
    gi = np.asarray(gate_idx).astype(np.int64)
    gs = np.asarray(gate_score, dtype=np.float32)
    W = np.asarray(W, dtype=np.float32)
    b = np.asarray(b, dtype=np.float32)

    P = T * TOP_K
    fe = gi.reshape(P)
    fg = gs.reshape(P)
    tok = np.arange(P) // TOP_K

    order = np.argsort(fe, kind="stable")
    counts = np.bincount(fe, minlength=NUM_EXPERT)
    starts = np.zeros(NUM_EXPERT + 1, np.int64)
    np.cumsum(counts, out=starts[1:])
    rank = np.arange(P) - starts[fe[order]]
    ok = rank < CAP
    sel = order[ok]
    rnk = rank[ok]

    xpad = np.zeros((NUM_EXPERT, CAP, D), np.float32)
    xpad[fe[sel], rnk] = inp[tok[sel]] * fg[sel, None]
    # flat chunk-major device layouts (must match _build_nc's chunking):
    # each chunk is [128, kl*D] with per-partition data contiguous
    xk = xpad.reshape(NUM_EXPERT, CAP, KT, 128)
    XH = KT // 2
    x_parts = [
        np.ascontiguousarray(
            xk[:, :, h * XH:(h + 1) * XH, :].transpose(0, 3, 2, 1)
        ).reshape(NUM_EXPERT, -1)
        for h in range(2)
    ]
    xt_dev = np.concatenate(x_parts, axis=1).astype(ml_dtypes.bfloat16)
    wk = W.reshape(NUM_EXPERT, KT, 128, D)
    w_parts = [
        np.ascontiguousarray(
            wk[:, k0:k0 + kl].transpose(0, 2, 1, 3)
        ).reshape(NUM_EXPERT, -1)
        for (k0, kl) in [(0, 2), (2, 2), (4, 2), (6, 2)]
    ]
    w_dev = np.concatenate(w_parts, axis=1).astype(ml_dtypes.bfloat16)

    if _NC is None:
        _NC = _build_nc()

    in_maps = [
        {"w": w_dev[c * EPC:(c + 1) * EPC],
         "xt": xt_dev[c * EPC:(c + 1) * EPC]}
        for c in range(N_CORES)
    ]
    res = run_bass_kernel_spmd(_NC, in_maps, list(range(N_CORES)),
                               trace=TRACE)
    LAST_RESULT = res
    zall = np.concatenate(
        [np.asarray(r["z"]).astype(np.float32) for r in res.results],
        axis=0)  # [E,CAP,D]

    zpairs = np.zeros((P, D), np.float32)
    zpairs[sel] = zall[fe[sel], rnk]
    # exact f32 fallback for over-capacity pairs (~2% of dispatches)
    overflow = order[~ok]
    if overflow.size:
        fe_o = fe[overflow]
        for e in np.unique(fe_o):
            pi = overflow[fe_o == e]
            zpairs[pi] = (inp[tok[pi]] * fg[pi, None]) @ W[e]

    y = zpairs.reshape(T, TOP_K, D).sum(axis=1)
    y += (gs[:, :, None] * b[gi]).sum(axis=1)
    return y.astype(np.float32)
